# revision 1
# baseline (speedup 1.0000x reference)
"""Trainium2 Bass kernel for involution-style aggregation (SAN Aggregation).

Per batch element b (one per NeuronCore, pure data parallel over B=8):
    out[c, p] = sum_{idx in 0..8} xshift_idx[c, p] * w[c % 16, idx, p]
with (i, j) = (idx // 3, idx % 3), zero padding 1, K=3, stride 1, p = oh*64+ow.

Design (fp16 compute, ~57.0us/core simulated, scale-relative error ~1.2e-3):
- Channels on partitions (2 blocks of 128), spatial flattened on free dim.
- The host pre-builds three column-shift versions of x (dj = -1, 0, +1 with
  zero fill) in fp16 ("xv"), so every tap reduces to a row-shifted window of
  one version: a contiguous, even-offset 2D access pattern that keeps every
  DVE tensor_tensor in the 2x 16-bit perf mode. Row clipping handles the
  vertical shifts (taps cover 63 or 64 full rows); column zero padding lives
  inside the shift versions.
- Weight replication across the 8 channel groups: the host lays w out as
  [128, 9*512] ("w", partition s*16 + wc holding spatial chunk s, taps in
  TAP_ORDER column blocks). Seven taps are broadcast on-chip with k=128
  selector matmuls (exact for a 0/1 selector) into PSUM [128, 1024] tiles
  and copied to fp16 wtap tiles by the Scalar engine. The last two taps'
  replicated weights come directly from the host ("wlate"), loaded
  mid-kernel when the DMA pipe is idle, shortening ACT's serial queue.
- Tap-sum: block 0 cols 0:2048 accumulate on the TensorEngine as identity
  matmuls into two persistent PSUM quarters (accumulation group opened by
  the first tap, closed by the last, drained by ACT). The remaining 6144
  columns accumulate via three independent in-place add half-chains on DVE.
  The last tap's multiplies are split into chain-aligned halves and the
  output DMAs are ordered by readiness, shortening the tail critical path
  (last product -> final adds -> final store).
- The 18 multiplies are split between DVE (slots in DVE_MULT_SLOTS) and
  GPSIMD so both engines finish together (DVE fp16 TT ~2.2us, GPSIMD ~3.4us
  per [128, 4096] op; all engine assignments tuned against the CoreSim
  cost model).
- Output is stored fp16 and upcast to f32 on the host.
- _legalize_sync_waits rewrites the scheduled IR so no instruction carries
  more than one sync wait (walrus codegen limit in this toolchain).
"""

import sys

for _p in (
    "/root/.axon_site",
    "/root/.axon_site/_ro/trn_rl_repo",
    "/root/.axon_site/_ro/pypackages",
):
    if _p not in sys.path:
        sys.path.append(_p)

from contextlib import ExitStack

import numpy as np

import concourse.bass as bass
import concourse.tile as tile
from concourse import mybir
from concourse.bass_utils import run_bass_kernel_spmd

B, C, H, W = 8, 256, 64, 64
WC, K2 = 16, 9
OH, OW = 64, 64
P = OH * OW
N_CORES = 8
F32 = mybir.dt.float32
F16 = mybir.dt.float16

# Tap order: center first so the first op fully initializes acc; then the
# other column-centered taps (j=1, same x version as center, already loaded),
# then j=0 taps, then j=2 taps — matching the x-version DMA order (1, 0, 2).
TAP_ORDER = [4, 1, 7, 0, 2, 3, 6, 8, 5]
# Both blocks' add chains run on DVE (2.17us/op, two independent chains
# interleave so chain latency never binds); GPSIMD is a pure multiply
# producer. A few multiplies stay on DVE to balance engine finish times;
# DVE_MULT_SLOTS picks which of the 16 non-center multiplies those are.
DVE_MULT_SLOTS = frozenset((1, 3, 6, 9, 12, 16))
# Engine per add chain: "v" = DVE, "g" = GPSIMD
CHAIN_ENGINES = {"b0h1": "v", "b1h0": "v", "b1h1": "v"}


def _legalize_sync_waits(nc, max_waits: int = 1) -> int:
    """Walrus codegen rejects instructions with >1 sync wait. Hoist excess
    waits onto same-engine drain carriers inserted just before the
    over-subscribed instruction (per-engine program order preserved)."""
    n_moved = 0
    counter = [0]
    for func in nc.m.functions:
        for bb in func.blocks:
            insts = list(bb.instructions)
            out = []
            changed = False
            for inst in insts:
                si = inst.sync_info
                waits = list(si.on_wait) if (si and si.on_wait) else []
                if len(waits) > max_waits:
                    extra, keep = waits[:-max_waits], waits[-max_waits:]
                    for w in extra:
                        counter[0] += 1
                        # NoOp, not Drain: carries the wait without flushing
                        # the engine pipeline
                        carrier = mybir.InstNoOp(
                            name=f"{inst.name}_wsplit{counter[0]}", ins=[], outs=[]
                        )
                        carrier.engine = inst.engine
                        carrier.sync_info = mybir.SyncInfo(on_wait=[w], on_update=[])
                        out.append(carrier)
                        n_moved += 1
                    si.on_wait = keep
                    changed = True
                out.append(inst)
            if changed:
                try:
                    bb.instructions = out
                except Exception:
                    cur = bb.instructions
                    cur[:] = out
    return n_moved


def _selector8() -> np.ndarray:
    """[128, 8*128] bank of selectors (bf16-exact 0/1 values). Selector s has
    sel[k, c] = 1 iff k == s*16 + c % 16: a k=128 matmul against the
    (s*16+wc)-partitioned weight buffer broadcasts spatial chunk s's weights
    to all 128 output channels."""
    sel = np.zeros((128, 8 * 128), dtype=np.float32)
    for s in range(8):
        for c in range(128):
            sel[s * WC + c % WC, s * 128 + c] = 1.0
    return sel


def _build(legalize: bool = True):
    nc = bass.Bass()
    # xv: [3, C, P] column-shift versions of x (dj = -1, 0, +1), fp16.
    xv = nc.declare_dram_parameter("xv", [3, C, P], F16, isOutput=False)
    # w pre-laid-out on host: [128, K2*512], partition (s*16 + wc) holds
    # w[wc, TAP_ORDER[ord], s*512:(s+1)*512] at columns ord*512.
    w = nc.declare_dram_parameter("w", [128, K2 * 512], F16, isOutput=False)
    # Last two taps' weights pre-replicated on host ([2, 128, P]): their
    # wtaps skip the PE->ACT broadcast and load mid-kernel when the DMA pipe
    # is otherwise idle, shortening ACT's serial wtap queue.
    wlate = nc.declare_dram_parameter("wlate", [2, 128, P], F16, isOutput=False)
    out = nc.declare_dram_parameter("out", [C, P], F16, isOutput=True)
    sel_np = np.concatenate(
        [_selector8(), np.eye(128, dtype=np.float32)], axis=1
    ).astype(np.float16)
    sel_d = nc.inline_tensor(sel_np, name="sel")

    with tile.TileContext(nc) as tc:
        with ExitStack() as ctx:
            selp = ctx.enter_context(tc.tile_pool(name="sel", bufs=1))
            xp = ctx.enter_context(tc.tile_pool(name="xb", bufs=1))
            wsp = ctx.enter_context(tc.tile_pool(name="wsb", bufs=1))
            wt = ctx.enter_context(tc.tile_pool(name="wt", bufs=3))
            ps = ctx.enter_context(tc.tile_pool(name="ps", bufs=2, space="PSUM"))
            pa = ctx.enter_context(tc.tile_pool(name="pa", bufs=1, space="PSUM"))
            tp = ctx.enter_context(tc.tile_pool(name="tmp", bufs=5))
            ap = ctx.enter_context(tc.tile_pool(name="acc", bufs=1))

            sel_t = selp.tile([128, 9 * 128], F16)
            nc.sync.dma_start(sel_t[:], sel_d[:])
            ident = sel_t[:, 8 * 128 : 9 * 128]
            # Persistent PSUM accumulators for block 0, cols 0:2048 — the
            # tap-sum for these columns runs on the TensorEngine as identity
            # matmuls with PSUM accumulation (start on first tap, stop on
            # last), freeing DVE/GPSIMD adds.
            ps_acc = [
                pa.tile([128, 1024], F32, tag=f"pacc{q}", name=f"pacc{q}")
                for q in range(2)
            ]
            # Warm the ACT engine's function table before the first real
            # PSUM->SBUF copy (the first Activation otherwise pays ~1.3us).
            warm = selp.tile([128, 2], F16, name="warm")
            nc.scalar.activation(
                warm[:], sel_t[:, 0:2], mybir.ActivationFunctionType.Copy
            )

            # Weight buffer: partition (s*16 + wc) holds w[wc, idx, s*512 +
            # 0:512] for each tap, in TAP_ORDER column blocks (ord*512).
            # The per-tap weight loads and per-(block, version) x loads are
            # interleaved on the single serialized DMA pipe so the
            # PE->ACT->DVE pipeline and both MAC engines ramp as early as
            # possible.
            wsb = wsp.tile([128, K2 * 512], F16)
            xt = []
            accs = []
            for blk in range(2):
                t = xp.tile([128, 3, P], F16, tag=f"xt{blk}", name=f"xt{blk}")
                xt.append(t)
                accs.append(ap.tile([128, P], F16, tag=f"acc{blk}", name=f"acc{blk}"))

            def load_w(lo, hi):
                nc.sync.dma_start(wsb[:, lo * 512 : hi * 512], w[:, lo * 512 : hi * 512])

            def load_x(blk, v):
                nc.sync.dma_start(xt[blk][:, v], xv[v, blk * 128 : (blk + 1) * 128])

            load_w(0, 1)
            load_x(0, 1)
            load_w(1, 3)
            load_x(1, 1)
            load_w(3, 6)
            load_x(0, 0)
            load_x(1, 0)
            load_w(6, 7)
            load_x(0, 2)
            load_x(1, 2)
            wlate_t = []
            for k in range(2):
                t = wt.tile([128, P], F16, tag=f"wlate{k}", name=f"wlate{k}")
                nc.sync.dma_start(t[:], wlate[k])
                wlate_t.append(t)

            n_mults = 0
            # per-chain state: (prev tmp AP, prev tap row offset) until the
            # first binary add writes acc
            chain_first = {"b0h1": None, "b1h0": None, "b1h1": None}
            for ord_, idx in enumerate(TAP_ORDER):
                # --- replicate w[:, idx, :] across the 8 channel groups ---
                if ord_ >= K2 - 2:
                    wtap = wlate_t[ord_ - (K2 - 2)]
                else:
                    wtap = wt.tile([128, P], F16)
                    for quarter in range(4):
                        pst = ps.tile([128, 1024], F32)
                        for ch in range(2):
                            s = quarter * 2 + ch
                            nc.tensor.matmul(
                                pst[:, ch * 512 : (ch + 1) * 512],
                                sel_t[:, s * 128 : (s + 1) * 128],
                                wsb[:, ord_ * 512 : (ord_ + 1) * 512],
                                start=True,
                                stop=True,
                            )
                        nc.scalar.activation(
                            wtap[:, quarter * 1024 : (quarter + 1) * 1024],
                            pst[:],
                            mybir.ActivationFunctionType.Copy,
                        )

                # --- tap geometry: row-shifted window of version v = j ---
                i, j = divmod(idx, 3)
                di = i - 1
                r0, rows = max(0, -di), OH - abs(di)
                n = rows * OW
                o0 = r0 * OW  # acc-plane offset of this tap's contribution
                tmps = []
                for blk in range(2):
                    xs = xt[blk][:, j, (r0 + di) * OW : (r0 + di) * OW + n]
                    wv = wtap[:, o0 : o0 + n]
                    mul_eng = nc.vector if n_mults in DVE_MULT_SLOTS else nc.gpsimd
                    n_mults += 1
                    tmp = tp.tile([128, P], F16)
                    if ord_ == K2 - 1:
                        # last tap: multiply in halves, the chain-consumed
                        # half first, so the tail adds unblock sooner
                        first = 2048 if blk == 0 else 0
                        for lo2 in (first, 2048 - first):
                            l2, h2 = max(lo2, 0), min(lo2 + 2048, n)
                            mul_eng.tensor_mul(
                                tmp[:, l2:h2], xs[:, l2:h2], wv[:, l2:h2]
                            )
                    else:
                        mul_eng.tensor_mul(tmp[:, 0:n], xs, wv)
                    tmps.append(tmp)

                # --- block 0, cols 0:2048: PE identity-matmul accumulation
                # into the two persistent PSUM quarters, in bank-aligned
                # 512-col chunks clipped to the tap's valid range (row-clipped
                # taps miss cols [0:64) or [4032:4096) of the plane; within
                # 0:2048 only the [0:64) clip matters). The center tap
                # (ord 0) covers everything and opens the accumulation group;
                # the last tap covers [0:4032) and closes it.
                for q in range(2):
                    qlo = q * 1024
                    for bank in range(2):
                        blo, bhi = qlo + bank * 512, qlo + (bank + 1) * 512
                        lo, hi = max(blo, o0), min(bhi, o0 + n)
                        if lo >= hi:
                            continue
                        nc.tensor.matmul(
                            ps_acc[q][:, lo - qlo : hi - qlo],
                            ident[:],
                            tmps[0][:, lo - o0 : hi - o0],
                            start=(ord_ == 0),
                            stop=(ord_ == K2 - 1),
                            skip_group_check=True,
                        )

                # --- block 0 cols 2048:4096 + block 1 (two half-chains):
                # independent add chains, engine per CHAIN_ENGINES
                for key, blk, alo, ahi in (
                    ("b0h1", 0, 2048, 4096),
                    ("b1h0", 1, 0, 2048),
                    ("b1h1", 1, 2048, 4096),
                ):
                    eng = nc.vector if CHAIN_ENGINES.get(key, "v") == "v" else nc.gpsimd
                    lo, hi = max(alo, o0), min(ahi, o0 + n)
                    av = accs[blk][:, lo:hi]
                    tv = tmps[blk][:, lo - o0 : hi - o0]
                    if chain_first[key] is None:
                        # center tap: stash; it covers the whole plane
                        chain_first[key] = tmps[blk]
                    elif chain_first[key] != "done":
                        ptmp = chain_first[key]
                        eng.tensor_add(av, ptmp[:, lo:hi], tv)
                        if lo > alo:
                            eng.tensor_copy(accs[blk][:, alo:lo], ptmp[:, alo:lo])
                        if hi < ahi:
                            eng.tensor_copy(accs[blk][:, hi:ahi], ptmp[:, hi:ahi])
                        chain_first[key] = "done"
                    else:
                        eng.tensor_add(av, av, tv)

            # drain the PSUM quarters into acc block 0 (f32 -> f16)
            for q in range(2):
                nc.scalar.activation(
                    accs[0][:, q * 1024 : (q + 1) * 1024],
                    ps_acc[q][:],
                    mybir.ActivationFunctionType.Copy,
                )

            # chain-produced regions flush as their chains complete; the
            # PSUM-drained half of block 0 goes last
            nc.sync.dma_start(out[0:128, 2048:4096], accs[0][:, 2048:4096])
            nc.sync.dma_start(out[0:128, 0:2048], accs[0][:, 0:2048])
            nc.sync.dma_start(out[128:256, 0:2048], accs[1][:, 0:2048])
            nc.sync.dma_start(out[128:256, 2048:4096], accs[1][:, 2048:4096])

    if legalize:
        _legalize_sync_waits(nc)
    return nc


_NC_CACHE = {}


def get_nc(legalize: bool = True):
    key = "nc_legal" if legalize else "nc_raw"
    if key not in _NC_CACHE:
        _NC_CACHE[key] = _build(legalize)
    return _NC_CACHE[key]


def _make_xv(x: np.ndarray) -> np.ndarray:
    """[3, C, P] fp16 column-shift versions of one batch element's x
    ([C, H, W] f32): version v reads x[., ., w + (v-1)] with zero fill."""
    xb = x.astype(np.float16)
    xvs = np.zeros((3, C, H, W), dtype=np.float16)
    xvs[0, :, :, 1:] = xb[:, :, :-1]  # v=0: dj=-1 -> x[., w-1]
    xvs[1] = xb
    xvs[2, :, :, :-1] = xb[:, :, 1:]  # v=2: dj=+1 -> x[., w+1]
    return xvs.reshape(3, C, P)


def _make_wsb(wb: np.ndarray) -> np.ndarray:
    """[128, K2*512] weight layout for one batch element ([WC, K2, P] fp16):
    partition (s*16 + wc) holds w[wc, TAP_ORDER[ord], s*512:(s+1)*512] at
    column block ord*512."""
    wt = wb[:, TAP_ORDER, :].reshape(WC, K2, 8, 512)
    return np.ascontiguousarray(
        wt.transpose(2, 0, 1, 3).reshape(128, K2 * 512)
    )


def kernel(x: np.ndarray, weight: np.ndarray) -> np.ndarray:
    x = np.ascontiguousarray(np.asarray(x, dtype=np.float32))
    weight = np.ascontiguousarray(np.asarray(weight, dtype=np.float32))
    assert x.shape == (B, C, H, W), x.shape
    assert weight.shape == (B, WC, K2, P), weight.shape

    nc = get_nc()
    wb = weight.astype(np.float16)
    in_maps = [
        {
            "xv": _make_xv(x[i]),
            "w": _make_wsb(wb[i]),
            "wlate": np.ascontiguousarray(
                np.stack(
                    [np.tile(wb[i][:, t, :], (8, 1)) for t in TAP_ORDER[-2:]]
                )
            ),
        }
        for i in range(N_CORES)
    ]
    try:
        res = run_bass_kernel_spmd(nc, in_maps, list(range(N_CORES)))
    except Exception:
        # the axon terminal occasionally reports a transient
        # NRT_EXEC_UNIT_UNRECOVERABLE for a known-good NEFF; retry once
        res = run_bass_kernel_spmd(nc, in_maps, list(range(N_CORES)))
    out = np.stack([res.results[i]["out"] for i in range(N_CORES)], axis=0)
    return out.reshape(B, C, H, W).astype(np.float32)



# revision 6
# speedup vs baseline: 1.0850x; 1.0850x over previous
"""Trainium2 Bass kernel for involution-style aggregation (SAN Aggregation).

Per batch element b (one per NeuronCore, pure data parallel over B=8):
    out[c, p] = sum_{idx in 0..8} x[c, p + 64*di + dj] * w[c % 16, idx, p]
with (di, dj) = (idx // 3 - 1, idx % 3 - 1), zero padding 1, K=3, stride 1,
p = oh*64 + ow flattened over the 64x64 output plane.

Design (fp16 compute):
- x is loaded ONCE per 128-channel block as [128, 4098] (one zero pad column
  each side).  Every tap is a flat-shifted window x[:, 1+delta+lo : 1+delta+hi]
  (delta = 64*di + dj); the cost model charges no penalty for odd offsets.
  Column-edge zero padding is folded into the WEIGHTS on the host: for dj=-1
  taps the weight at output columns p%64==0 is zeroed (and p%64==63 for
  dj=+1), so the wrapped x values multiply by zero.  Row-edge padding is
  handled by clipping each tap to its valid row range.
- Weight replication across the 8 channel groups is done by the DMA engines:
  a single dma_start per (tap, half) with a stride-0 leading source dim reads
  w[:, tap, half] ([16, 2048] in HBM) and writes the replicated [128, 2048]
  SBUF tile directly in fp16.  No PE broadcast, no PSUM staging, no ACT
  drains for weights.
- The 36 half-plane multiplies (9 taps x 2 channel blocks x 2 column halves)
  are split between DVE (~1.13us each) and GPSIMD (~1.71us each), balanced
  greedily so both engines finish together (~24us).  This is the multiply
  roofline for the two tensor-tensor-capable engines.
- Accumulation over the 9 taps costs nothing on DVE/GPSIMD:
  * Three half-regions (block0 lo, block0 hi, block1 lo) accumulate on the
    TensorEngine as identity matmuls into [128, 2048] f32 PSUM tiles
    (ping-pong, 4 banks each).  Center tap opens the accumulation group,
    the last tap (chosen to cover the full region) closes it; row-clipped
    taps accumulate bank-aligned clipped chunks.  ACT drains each finished
    region f32->f16.
  * The fourth half-region (block1 hi) accumulates via gpsimd-issued
    accumulating DMAs (SWDGE cce add) on the otherwise idle DMA queue: the
    center tap's multiply writes the acc tile directly, the other eight taps
    are acc += tmp copies (~2.2us each, fully off the compute engines).
- DMA is spread over the three queues (SP/sync, Activation/scalar, Pool/
  gpsimd), which the cost model executes in parallel: x halves + early wtaps
  feed the first multiplies within ~2us.
- Output is stored fp16 and upcast to f32 on the host.
- _legalize_sync_waits rewrites the scheduled IR so no instruction carries
  more than one sync wait (walrus codegen limit in this toolchain).
"""

import sys

for _p in (
    "/root/.axon_site",
    "/root/.axon_site/_ro/trn_rl_repo",
    "/root/.axon_site/_ro/pypackages",
):
    if _p not in sys.path:
        sys.path.append(_p)

from contextlib import ExitStack

import numpy as np

import concourse.bass as bass
import concourse.tile as tile
from concourse import mybir
from concourse.bass_utils import run_bass_kernel_spmd

B, C, H, W = 8, 256, 64, 64
WC, K2 = 16, 9
OH, OW = 64, 64
P = OH * OW
PP = P + 2  # padded x row length
N_CORES = 8
F32 = mybir.dt.float32
F16 = mybir.dt.float16
HALF = P // 2  # 2048

# Tap geometry: idx -> (di, dj), delta, valid output range [o0, o0+n)
def _tap_geom(idx):
    di, dj = idx // 3 - 1, idx % 3 - 1
    delta = 64 * di + dj
    r0 = max(0, -di)
    rows = OH - abs(di)
    return delta, r0 * OW, rows * OW


# Region tap orders: center tap (4) first (it covers every region fully and
# opens the PSUM accumulation group); the LAST tap must also fully cover the
# region so stop=True closes every bank's group.
#   lo half [0:2048): di=+1 taps (6,7,8) cover [0:4032) -> any tap covers lo
#     except di=-1 taps (0,1,2) which miss [0:64). Last must be di>=0.
#   hi half [2048:4096): di=-1 taps cover [64:4096); di=+1 miss [4032:).
#     Last must be di<=0.
ORDER_L = [4, 0, 1, 2, 6, 8, 3, 5, 7]  # last di=+1? 7 -> di=+1 covers [0:4032) >= lo ✓
ORDER_H = [4, 6, 7, 8, 0, 2, 3, 5, 1]  # last 1 -> di=-1 covers [64:4096) >= hi ✓
ORDER_R3 = [4, 6, 7, 8, 0, 2, 3, 5, 1]  # dma-add region (block1 hi)

# Global multiply-job order: (region, pos-in-order). R0/R3 interleaved first,
# then R1, then R2 (matching PSUM region lifetimes R0 -> R1 -> R2).
#   R0 = block0 lo (PE), R1 = block0 hi (PE), R2 = block1 lo (PE),
#   R3 = block1 hi (DMA-add).
REGIONS = {
    "R0": dict(blk=0, lo=0, hi=HALF, kind="pe", order=ORDER_L),
    "R1": dict(blk=0, lo=HALF, hi=P, kind="pe", order=ORDER_H),
    "R2": dict(blk=1, lo=0, hi=HALF, kind="pe", order=ORDER_L),
    "R3": dict(blk=1, lo=HALF, hi=P, kind="dma", order=ORDER_R3),
}
JOB_SEQ = (
    [("R3", 0), ("R0", 0)]
    + [x for k in range(1, 9) for x in (("R0", k), ("R3", k))]
    + [("R1", k) for k in range(9)]
    + [("R2", k) for k in range(9)]
)


def _legalize_sync_waits(nc, max_waits: int = 1) -> int:
    """Walrus codegen rejects instructions with >1 sync wait. Hoist excess
    waits onto same-engine NoOp carriers inserted just before the
    over-subscribed instruction (per-engine program order preserved)."""
    n_moved = 0
    counter = [0]
    for func in nc.m.functions:
        for bb in func.blocks:
            insts = list(bb.instructions)
            out = []
            changed = False
            for inst in insts:
                si = inst.sync_info
                waits = list(si.on_wait) if (si and si.on_wait) else []
                if len(waits) > max_waits:
                    extra, keep = waits[:-max_waits], waits[-max_waits:]
                    for w in extra:
                        counter[0] += 1
                        carrier = mybir.InstNoOp(
                            name=f"{inst.name}_wsplit{counter[0]}", ins=[], outs=[]
                        )
                        carrier.engine = inst.engine
                        carrier.sync_info = mybir.SyncInfo(on_wait=[w], on_update=[])
                        out.append(carrier)
                        n_moved += 1
                    si.on_wait = keep
                    changed = True
                out.append(inst)
            if changed:
                try:
                    bb.instructions = out
                except Exception:
                    cur = bb.instructions
                    cur[:] = out
    return n_moved


def _build(legalize: bool = True):
    nc = bass.Bass()
    # x2: [2, 128, PP] fp16, one zero column padded each side of each block.
    x2 = nc.declare_dram_parameter("x2", [2, 128, PP], F16, isOutput=False)
    # w9: [K2, WC, P] fp16 with column-edge zeroing folded in.
    w9 = nc.declare_dram_parameter("w9", [K2, WC, P], F16, isOutput=False)
    out = nc.declare_dram_parameter("out", [2, 128, P], F16, isOutput=True)
    ident_np = np.eye(128, dtype=np.float16)
    ident_d = nc.inline_tensor(ident_np, name="ident")

    with tile.TileContext(nc) as tc:
        with ExitStack() as ctx:
            idp = ctx.enter_context(tc.tile_pool(name="idp", bufs=1))
            xp = ctx.enter_context(tc.tile_pool(name="xp", bufs=1))
            wp = ctx.enter_context(tc.tile_pool(name="wp", bufs=1))
            tp = ctx.enter_context(tc.tile_pool(name="tp", bufs=8))
            pa = ctx.enter_context(tc.tile_pool(name="pa", bufs=1, space="PSUM"))
            op = ctx.enter_context(tc.tile_pool(name="op", bufs=1))

            ident = idp.tile([128, 128], F16, name="ident_t")
            nc.sync.dma_start(ident[:], ident_d[:])
            # Warm ACT's function table before the first real drain.
            warm = idp.tile([128, 2], F16, name="warm")
            nc.scalar.activation(
                warm[:], ident[:, 0:2], mybir.ActivationFunctionType.Copy
            )

            xt = [xp.tile([128, PP], F16, tag=f"x{b}", name=f"x{b}") for b in (0, 1)]
            # wtap half tiles, one per (tap, half)
            wt = {}
            for t in range(K2):
                for h in (0, 1):
                    wt[(t, h)] = wp.tile(
                        [128, HALF], F16, tag=f"w{t}_{h}", name=f"w{t}_{h}"
                    )

            def load_w(queue, t, h):
                src = w9[t, :, h * HALF : (h + 1) * HALF]
                src = src.unsqueeze(0).broadcast_to([8, WC, HALF])
                queue.dma_start(wt[(t, h)][:], src)

            def load_x(queue, b, c0, c1):
                queue.dma_start(xt[b][:, c0:c1], x2[b, :, c0:c1])

            # --- input DMA schedule -------------------------------------
            # sync queue: block0 x + lo-half wtaps for R0 (order ORDER_L) +
            #   R1's hi-half wtaps later.
            # scalar queue: block1 x (hi part first for R3) + hi-half wtaps
            #   for R3 (order ORDER_R3), then remaining lo-half wtaps for R2.
            # First R0 job needs x0 cols [<=2114) and w(4,0); first R3 job
            # needs x1 cols [1984:4098) and w(4,1).
            load_x(nc.sync, 0, 0, 2176)
            load_w(nc.sync, 4, 0)
            load_x(nc.scalar, 1, 1984, PP)
            load_w(nc.scalar, 4, 1)
            # R0's next lo wtaps on sync; R3's next hi wtaps on scalar.
            sync_seq = [(t, 0) for t in ORDER_L[1:]]
            scal_seq = [(t, 1) for t in ORDER_R3[1:]]
            # R1 needs hi wtaps (same tiles as R3's — already loaded);
            # R2 needs lo wtaps (same as R0's). So 18 wtap loads total.
            load_x(nc.sync, 0, 2176, PP)
            load_x(nc.scalar, 1, 0, 1984)
            for i in range(max(len(sync_seq), len(scal_seq))):
                if i < len(sync_seq):
                    load_w(nc.sync, *sync_seq[i])
                if i < len(scal_seq):
                    load_w(nc.scalar, *scal_seq[i])

            # --- compute ------------------------------------------------
            # Greedy DVE/GPSIMD balance of the 36 half-multiplies.
            eng_busy = {"v": 0.0, "g": 0.0}
            ECOST = {"v": 0.55, "g": 0.8333}  # ns per col per partition-row

            # two PSUM half-plane tiles (4 banks each): R0 and R2 share
            # "ping" (R2 starts after R0's drain frees it), R1 uses "pong".
            ps_ping = pa.tile([128, HALF], F32, tag="ps_ping", name="ps_ping")
            ps_pong = pa.tile([128, HALF], F32, tag="ps_pong", name="ps_pong")
            psum = {"R0": ps_ping, "R1": ps_pong, "R2": ps_ping}
            acc3 = op.tile([128, HALF], F16, tag="acc3", name="acc3")
            ob = {
                "R0": op.tile([128, HALF], F16, tag="ob0", name="ob0"),
                "R1": op.tile([128, HALF], F16, tag="ob1", name="ob1"),
                "R2": op.tile([128, HALF], F16, tag="ob2", name="ob2"),
            }

            njobs_done = {r: 0 for r in REGIONS}
            for rname, k in JOB_SEQ:
                R = REGIONS[rname]
                t = R["order"][k]
                delta, o0, n = _tap_geom(t)
                alo, ahi = R["lo"], R["hi"]
                lo, hi = max(alo, o0), min(ahi, o0 + n)
                assert lo < hi
                blk = R["blk"]
                h = alo // HALF
                wv = wt[(t, h)][:, lo - alo : hi - alo]
                xv = xt[blk][:, 1 + delta + lo : 1 + delta + hi]
                cols = hi - lo

                # pick engine greedily by projected finish time
                ev = eng_busy["v"] + cols * ECOST["v"]
                eg = eng_busy["g"] + cols * ECOST["g"]
                eng = "v" if ev <= eg else "g"
                eng_busy[eng] = min(ev, eg)
                mul_eng = nc.vector if eng == "v" else nc.gpsimd

                if R["kind"] == "dma":
                    if k == 0:
                        # center tap covers the whole region: multiply
                        # straight into the acc tile
                        mul_eng.tensor_mul(acc3[:, lo - alo : hi - alo], xv, wv)
                    else:
                        tmp = tp.tile([128, HALF], F16)
                        mul_eng.tensor_mul(tmp[:, 0 : hi - lo], xv, wv)
                        nc.gpsimd.dma_start(
                            acc3[:, lo - alo : hi - alo],
                            tmp[:, 0 : hi - lo],
                            accum_op=mybir.AluOpType.add,
                        )
                else:
                    tmp = tp.tile([128, HALF], F16)
                    mul_eng.tensor_mul(tmp[:, 0 : hi - lo], xv, wv)
                    # identity-matmul accumulate in bank-aligned 512 chunks
                    is_first = k == 0
                    is_last = k == K2 - 1
                    for bank in range(4):
                        blo = alo + bank * 512
                        bhi = blo + 512
                        l2, h2 = max(blo, lo), min(bhi, hi)
                        if l2 >= h2:
                            continue
                        nc.tensor.matmul(
                            psum[rname][:, l2 - alo : h2 - alo],
                            ident[:],
                            tmp[:, l2 - lo : h2 - lo],
                            start=is_first,
                            stop=is_last,
                            skip_group_check=True,
                        )
                njobs_done[rname] += 1
                if njobs_done[rname] == K2 and R["kind"] == "pe":
                    # region complete: drain psum f32 -> f16 and store
                    nc.scalar.activation(
                        ob[rname][:],
                        psum[rname][:],
                        mybir.ActivationFunctionType.Copy,
                    )
                    q = nc.sync if rname in ("R0", "R1") else nc.scalar
                    q.dma_start(out[blk, :, alo:ahi], ob[rname][:])

            # R3 store (after its 8 accum DMAs)
            nc.scalar.dma_start(out[1, :, HALF:P], acc3[:])

    if legalize:
        _legalize_sync_waits(nc)
    return nc


_NC_CACHE = {}


def get_nc(legalize: bool = True):
    key = "nc_legal" if legalize else "nc_raw"
    if key not in _NC_CACHE:
        _NC_CACHE[key] = _build(legalize)
    return _NC_CACHE[key]


def _make_x2(xb: np.ndarray) -> np.ndarray:
    """[2, 128, PP] fp16 padded view of one batch element's x ([C, H, W])."""
    xf = xb.reshape(C, P).astype(np.float16)
    xp = np.zeros((2, 128, PP), dtype=np.float16)
    xp[0, :, 1 : P + 1] = xf[0:128]
    xp[1, :, 1 : P + 1] = xf[128:256]
    return xp


def _make_w9(wb: np.ndarray) -> np.ndarray:
    """[K2, WC, P] fp16 weights with column-edge zeroing folded in:
    dj=-1 taps zero output columns p%64==0, dj=+1 taps zero p%64==63."""
    w = np.ascontiguousarray(wb.transpose(1, 0, 2)).astype(np.float16)
    w = w.reshape(K2, WC, OH, OW).copy()
    for idx in range(K2):
        dj = idx % 3 - 1
        if dj == -1:
            w[idx, :, :, 0] = 0
        elif dj == 1:
            w[idx, :, :, OW - 1] = 0
    return w.reshape(K2, WC, P)


def kernel(x: np.ndarray, weight: np.ndarray) -> np.ndarray:
    x = np.ascontiguousarray(np.asarray(x, dtype=np.float32))
    weight = np.ascontiguousarray(np.asarray(weight, dtype=np.float32))
    assert x.shape == (B, C, H, W), x.shape
    assert weight.shape == (B, WC, K2, P), weight.shape

    nc = get_nc()
    in_maps = [
        {
            "x2": _make_x2(x[i]),
            "w9": _make_w9(weight[i].astype(np.float16)),
        }
        for i in range(N_CORES)
    ]
    try:
        res = run_bass_kernel_spmd(nc, in_maps, list(range(N_CORES)))
    except Exception:
        # the axon terminal occasionally reports a transient
        # NRT_EXEC_UNIT_UNRECOVERABLE for a known-good NEFF; retry once
        res = run_bass_kernel_spmd(nc, in_maps, list(range(N_CORES)))
    out = np.stack([res.results[i]["out"] for i in range(N_CORES)], axis=0)
    return out.reshape(B, C, H, W).astype(np.float32)


# revision 8
# speedup vs baseline: 1.3410x; 1.2359x over previous
"""Trainium2 Bass kernel for involution-style aggregation (SAN Aggregation).

Per batch element b (one per NeuronCore, pure data parallel over B=8):
    out[c, p] = sum_{idx in 0..8} x[c, p + 64*di + dj] * w[c % 16, idx, p]
with (di, dj) = (idx // 3 - 1, idx % 3 - 1), zero padding 1, K=3, stride 1.

Layout (the key trick): SBUF partition q = wc*8 + rb packs the 16 weight
channels x 8 row-blocks; the free dim per 128-channel block is
m = rr*512 + g*64 + j (rr = row-in-rowblock 0..7, g = channel group 0..7,
j = column 0..63).  In this layout the weight for output (q, rr, g, j) is
wt[q, rr*64 + j] — a COMPACT [128, 512] tile per tap consumed via a
stride-0 broadcast access pattern ([[64,nrr],[0,8],[1,64]]), so weights are
never replicated (1.2 MB total DMA instead of 9 MB, and no PE broadcast).

- Row shifts di stay inside a partition for rr+di in [0,8) (flat offset
  512*di + dj); the one boundary row per row-block reads from small
  staged tiles xup/xdn ([128, 514] per block) whose rb=0 / rb=7 rows are
  zero, so row-edge padding needs no clipping at all.  Column-edge padding
  is folded into the weights host-side (dj=-1 taps zero j==0, dj=+1 taps
  zero j==63), so wrapped reads multiply by zero.  Every tap therefore
  contributes a full densely-written plane.
- The ~73.7K column-multiplies split between DVE (0.52 ns/col, fp16 2x
  mode) and GPSIMD (0.83 ns/col), greedily balanced -> ~24.5us each, the
  two-engine multiply roofline.
- Tap accumulation is offloaded: regions A=(blk0,lo), B=(blk0,hi),
  C=(blk1,lo) accumulate on the TensorEngine as 512-wide identity matmuls
  into [128,2048] f32 PSUM tiles; region D=(blk1,hi) takes K_PE taps on PE
  (its own PSUM tile, drained mid-kernel by ACT) and the rest as f16
  quarter-adds on DVE/GPSIMD.  Phasing keeps <=2 PSUM region tiles live:
  phase 1 computes A and D, phase 2 computes B and C.
- DMA is issued from both SP (sync) and ACT (scalar) queues, which execute
  in parallel in this cost model; DMA occupies the issuing engine, so the
  compact-weight layout is what keeps the queues short.
- Output is stored fp16 in the permuted layout and un-permuted on the host.
- _legalize_sync_waits rewrites the scheduled IR so no instruction carries
  more than one sync wait (walrus codegen limit in this toolchain).
"""

import sys

for _p in (
    "/root/.axon_site",
    "/root/.axon_site/_ro/trn_rl_repo",
    "/root/.axon_site/_ro/pypackages",
):
    if _p not in sys.path:
        sys.path.append(_p)

from contextlib import ExitStack

import numpy as np

import concourse.bass as bass
import concourse.tile as tile
from concourse import mybir
from concourse.bass_utils import run_bass_kernel_spmd

B, C, H, W = 8, 256, 64, 64
WC, K2 = 16, 9
OH, OW = 64, 64
P = OH * OW
N_CORES = 8
F32 = mybir.dt.float32
F16 = mybir.dt.float16
HALF = P // 2  # 2048
QUAR = P // 4  # 1024

# tap processing order within each phase: di=0 first (no xe dependency),
# then di=+1 (needs xdn), then di=-1 (needs xup)
TAP_ORDER = [4, 3, 5, 7, 6, 8, 1, 0, 2]
# number of region-D taps accumulated on PE before the mid-kernel drain
K_PE = 4
# engine cost constants for the greedy DVE/GPSIMD balance
EV, EG = 0.5208, 0.8333
EV_FIX = 60.0


def _tap_geom(idx):
    di, dj = idx // 3 - 1, idx % 3 - 1
    return di, dj


def _legalize_sync_waits(nc, max_waits: int = 1) -> int:
    """Walrus codegen rejects instructions with >1 sync wait. Hoist excess
    waits onto same-engine NoOp carriers inserted just before the
    over-subscribed instruction (per-engine program order preserved)."""
    n_moved = 0
    counter = [0]
    for func in nc.m.functions:
        for bb in func.blocks:
            insts = list(bb.instructions)
            out = []
            changed = False
            for inst in insts:
                si = inst.sync_info
                waits = list(si.on_wait) if (si and si.on_wait) else []
                if len(waits) > max_waits:
                    extra, keep = waits[:-max_waits], waits[-max_waits:]
                    for w in extra:
                        counter[0] += 1
                        carrier = mybir.InstNoOp(
                            name=f"{inst.name}_wsplit{counter[0]}", ins=[], outs=[]
                        )
                        carrier.engine = inst.engine
                        carrier.sync_info = mybir.SyncInfo(on_wait=[w], on_update=[])
                        out.append(carrier)
                        n_moved += 1
                    si.on_wait = keep
                    changed = True
                out.append(inst)
            if changed:
                try:
                    bb.instructions = out
                except Exception:
                    cur = bb.instructions
                    cur[:] = out
    return n_moved


def _build(legalize: bool = True):
    nc = bass.Bass()
    xb_d = nc.declare_dram_parameter("xb", [2, 128, P + 2], F16, isOutput=False)
    xe_d = nc.declare_dram_parameter("xe", [2, 2, 128, 514], F16, isOutput=False)
    wt_d = nc.declare_dram_parameter("wt", [K2, 128, 512], F16, isOutput=False)
    out = nc.declare_dram_parameter("out", [2, 128, P], F16, isOutput=True)
    ident_d = nc.inline_tensor(np.eye(128, dtype=np.float16), name="ident")

    eng_busy = {"v": 0.0, "g": 0.0}

    def pick_engine(nels):
        ev = eng_busy["v"] + nels * EV + EV_FIX
        eg = eng_busy["g"] + nels * EG
        if ev <= eg:
            eng_busy["v"] = ev
            return nc.vector
        eng_busy["g"] = eg
        return nc.gpsimd

    with tile.TileContext(nc) as tc:
        with ExitStack() as ctx:
            idp = ctx.enter_context(tc.tile_pool(name="idp", bufs=1))
            xp = ctx.enter_context(tc.tile_pool(name="xp", bufs=1))
            wp = ctx.enter_context(tc.tile_pool(name="wp", bufs=1))
            tp = ctx.enter_context(tc.tile_pool(name="tp", bufs=6))
            tp2 = ctx.enter_context(tc.tile_pool(name="tp2", bufs=4))
            pa = ctx.enter_context(tc.tile_pool(name="pa", bufs=1, space="PSUM"))
            op = ctx.enter_context(tc.tile_pool(name="op", bufs=1))

            ident = idp.tile([128, 128], F16, name="ident_t")
            warm = idp.tile([128, 2], F16, name="warm")

            xt = [xp.tile([128, P + 2], F16, tag=f"x{b}", name=f"x{b}") for b in (0, 1)]
            xe = {}
            for d in (0, 1):  # 0=up, 1=dn
                for b in (0, 1):
                    xe[(d, b)] = xp.tile(
                        [128, 514], F16, tag=f"xe{d}{b}", name=f"xe{d}{b}"
                    )
            wt = [
                wp.tile([128, 512], F16, tag=f"w{t}", name=f"w{t}") for t in range(K2)
            ]

            # --- input DMA schedule (SP + ACT queues run in parallel) ----
            # SP: xb0 (phase-1 region A needs cols <= ~2626 first), weights,
            #     rest of xb0.
            # ACT: xb1 hi-part (region D), xe tiles, xb1 lo-part.
            nc.sync.dma_start(xt[0][:, 0:2626], xb_d[0, :, 0:2626])
            nc.sync.dma_start(wt[4][:], wt_d[4])
            nc.scalar.dma_start(xt[1][:, 1536 : P + 2], xb_d[1, :, 1536 : P + 2])
            nc.scalar.dma_start(ident[:], ident_d[:])
            for t in (3, 5, 7):
                nc.sync.dma_start(wt[t][:], wt_d[t])
            nc.scalar.dma_start(xe[(1, 1)][:], xe_d[1, 1])  # dn, blk1
            nc.scalar.dma_start(xe[(1, 0)][:], xe_d[1, 0])  # dn, blk0
            for t in (6, 8, 1):
                nc.sync.dma_start(wt[t][:], wt_d[t])
            nc.scalar.dma_start(xe[(0, 1)][:], xe_d[0, 1])  # up, blk1
            nc.scalar.dma_start(xe[(0, 0)][:], xe_d[0, 0])  # up, blk0
            for t in (0, 2):
                nc.sync.dma_start(wt[t][:], wt_d[t])
            nc.sync.dma_start(xt[0][:, 2626 : P + 2], xb_d[0, :, 2626 : P + 2])
            nc.scalar.dma_start(xt[1][:, 0:1536], xb_d[1, :, 0:1536])
            nc.scalar.activation(
                warm[:], ident[:, 0:2], mybir.ActivationFunctionType.Copy
            )

            # --- helpers ------------------------------------------------
            def w_ap(t, rr_lo, rr_hi):
                """weight AP [[64, nrr], [0, 8], [1, 64]] at row offset."""
                w3 = wt[t][:].rearrange("p (rr j) -> p rr j", rr=8)
                nrr = rr_hi - rr_lo
                return (
                    w3[:, rr_lo:rr_hi]
                    .unsqueeze(2)
                    .broadcast_to([128, nrr, 8, 64])
                )

            def grid3(ap2):
                """[128, n*512] -> [128, n, 8, 64]"""
                return ap2.rearrange("p (rr g j) -> p rr g j", g=8, j=64)

            def mult_main(dst2, blk, t, olo, ohi):
                """dst2 <- x_window * w over output flat range [olo, ohi)
                (multiple of 512), rows rr = olo//512 .. ohi//512."""
                di, dj = _tap_geom(t)
                s = 1 + olo + 512 * di + dj
                xap = grid3(xt[blk][:, s : s + (ohi - olo)])
                oap = grid3(dst2)
                wap = w_ap(t, olo // 512, ohi // 512)
                pick_engine(ohi - olo).tensor_mul(oap, xap, wap)

            def mult_boundary(dst2, blk, t):
                """the one boundary row-in-rowblock (rr=0 for di=-1, rr=7
                for di=+1); dst2 is the [128, 512] output slice."""
                di, dj = _tap_geom(t)
                src = xe[(0 if di < 0 else 1, blk)]
                rr_b = 0 if di < 0 else 7
                xap = src[:, 1 + dj : 1 + dj + 512].rearrange(
                    "p (g j) -> p g j", g=8
                )
                oap = dst2.rearrange("p (g j) -> p g j", g=8)
                w3 = wt[t][:].rearrange("p (rr j) -> p rr j", rr=8)
                wap = w3[:, rr_b].unsqueeze(1).broadcast_to([128, 8, 64])
                pick_engine(512).tensor_mul(oap, xap, wap)

            def region_ops(pool, blk, t, alo, ahi):
                """emit multiply op(s) producing this tap's [alo, ahi)
                half-plane into a fresh [128, 2048] tile; returns it."""
                di, _ = _tap_geom(t)
                tmp = pool.tile([128, HALF], F16)
                if di == 0:
                    mult_main(tmp[:, 0:HALF], blk, t, alo, ahi)
                elif di == -1:
                    if alo == 0:
                        mult_boundary(tmp[:, 0:512], blk, t)
                        mult_main(tmp[:, 512:HALF], blk, t, 512, ahi)
                    else:
                        mult_main(tmp[:, 0:HALF], blk, t, alo, ahi)
                else:  # di == +1
                    if ahi == P:
                        mult_main(tmp[:, 0 : HALF - 512], blk, t, alo, P - 512)
                        mult_boundary(tmp[:, HALF - 512 : HALF], blk, t)
                    else:
                        mult_main(tmp[:, 0:HALF], blk, t, alo, ahi)
                return tmp

            def pe_acc(psum_t, tmp, first, last):
                for c0 in range(0, HALF, 512):
                    nc.tensor.matmul(
                        psum_t[:, c0 : c0 + 512],
                        ident[:],
                        tmp[:, c0 : c0 + 512],
                        start=first,
                        stop=last,
                        skip_group_check=True,
                    )

            # two [128,2048] f32 PSUM tiles = 8 banks; B reuses D's banks
            # after D's mid-phase-1 drain, C reuses A's after A's drain.
            ps_a = pa.tile([128, HALF], F32, tag="ps_a", name="ps_a")
            ps_d = pa.tile([128, HALF], F32, tag="ps_d", name="ps_d")
            ps_b = pa.tile([128, HALF], F32, tag="ps_d", name="ps_b")
            ob = {
                r: op.tile([128, HALF], F16, tag=f"ob_{r}", name=f"ob_{r}")
                for r in ("A", "B", "C", "D")
            }

            # --- phase 1: regions A (blk0 lo, PE) + D (blk1 hi, PE/adds) -
            d_pe_taps = set(TAP_ORDER[1 : 1 + K_PE])
            d_add_taps = [t for t in TAP_ORDER[1 + K_PE :]]
            d_drained = False
            for pos, t in enumerate(TAP_ORDER):
                tmp_a = region_ops(tp, 0, t, 0, HALF)
                pe_acc(ps_a, tmp_a, pos == 0, pos == K2 - 1)
                if pos == 0 or t in d_pe_taps:
                    tmp_d = region_ops(tp2, 1, t, HALF, P)
                    pe_acc(ps_d, tmp_d, pos == 0, t == TAP_ORDER[K_PE])
                    if t == TAP_ORDER[K_PE]:
                        # close + drain the partial D sum mid-kernel
                        nc.scalar.activation(
                            ob["D"][:], ps_d[:], mybir.ActivationFunctionType.Copy
                        )
                        d_drained = True
                elif d_drained:
                    tmp_d = region_ops(tp2, 1, t, HALF, P)
                    for q0 in (0, QUAR):
                        pick_engine(QUAR).tensor_add(
                            ob["D"][:, q0 : q0 + QUAR],
                            ob["D"][:, q0 : q0 + QUAR],
                            tmp_d[:, q0 : q0 + QUAR],
                        )
                else:
                    # defer: should not happen (add taps come after drain)
                    raise AssertionError("D add tap before drain")
            nc.scalar.activation(
                ob["A"][:], ps_a[:], mybir.ActivationFunctionType.Copy
            )
            nc.sync.dma_start(out[0, :, 0:HALF], ob["A"][:])
            nc.scalar.dma_start(out[1, :, HALF:P], ob["D"][:])

            # --- phase 2: regions B (blk0 hi, PE) + C (blk1 lo, PE) ------
            ps_c = pa.tile([128, HALF], F32, tag="ps_a", name="ps_c")
            for pos, t in enumerate(TAP_ORDER):
                tmp_b = region_ops(tp, 0, t, HALF, P)
                pe_acc(ps_b, tmp_b, pos == 0, pos == K2 - 1)
                tmp_c = region_ops(tp2, 1, t, 0, HALF)
                pe_acc(ps_c, tmp_c, pos == 0, pos == K2 - 1)
            nc.scalar.activation(
                ob["B"][:], ps_b[:], mybir.ActivationFunctionType.Copy
            )
            nc.sync.dma_start(out[0, :, HALF:P], ob["B"][:])
            nc.scalar.activation(
                ob["C"][:], ps_c[:], mybir.ActivationFunctionType.Copy
            )
            nc.scalar.dma_start(out[1, :, 0:HALF], ob["C"][:])

    if legalize:
        _legalize_sync_waits(nc)
    return nc


_NC_CACHE = {}


def get_nc(legalize: bool = True):
    key = "nc_legal" if legalize else "nc_raw"
    if key not in _NC_CACHE:
        _NC_CACHE[key] = _build(legalize)
    return _NC_CACHE[key]


# ---- host-side layout helpers ------------------------------------------


def _make_xb(xb: np.ndarray) -> np.ndarray:
    """[2, 128, P+2] fp16: xb[blk, wc*8+rb, 1 + rr*512 + g*64 + j] =
    x[(blk*8+g)*16 + wc, (rb*8+rr)*64 + j]; zero pad columns 0 and P+1."""
    xf = xb.reshape(2, 8, 16, 8, 8, 64)  # (blk, g, wc, rb, rr, j)
    perm = xf.transpose(0, 2, 3, 4, 1, 5).reshape(2, 128, P)
    outp = np.zeros((2, 128, P + 2), dtype=np.float16)
    outp[:, :, 1 : P + 1] = perm
    return outp


def _make_xe(xb: np.ndarray) -> np.ndarray:
    """[2(dir), 2(blk), 128, 514] fp16 boundary-row tiles:
    dir 0 (up): row rb*8 - 1 (zeros for rb == 0)
    dir 1 (dn): row rb*8 + 8 (zeros for rb == 7)"""
    x4 = xb.reshape(2, 8, 16, 64, 64)  # (blk, g, wc, r, j)
    res = np.zeros((2, 2, 16, 8, 8, 64), dtype=np.float16)  # (dir,blk,wc,rb,g,j)
    for rb in range(8):
        if rb > 0:
            res[0, :, :, rb] = x4[:, :, :, rb * 8 - 1, :].transpose(0, 2, 1, 3)
        if rb < 7:
            res[1, :, :, rb] = x4[:, :, :, rb * 8 + 8, :].transpose(0, 2, 1, 3)
    out = np.zeros((2, 2, 128, 514), dtype=np.float16)
    out[:, :, :, 1:513] = res.reshape(2, 2, 128, 512)
    return out


def _make_wt(wb: np.ndarray) -> np.ndarray:
    """[K2, 128, 512] fp16: wt[tap, wc*8+rb, rr*64+j] = w[wc, tap,
    (rb*8+rr)*64+j], with column-edge zeroing folded in."""
    w = wb.astype(np.float16).reshape(WC, K2, OH, OW).copy()
    for idx in range(K2):
        dj = idx % 3 - 1
        if dj == -1:
            w[:, idx, :, 0] = 0
        elif dj == 1:
            w[:, idx, :, OW - 1] = 0
    w = w.reshape(WC, K2, 8, 8, 64).transpose(1, 0, 2, 3, 4)
    return np.ascontiguousarray(w.reshape(K2, 128, 512))


def _unperm_out(op_: np.ndarray) -> np.ndarray:
    """inverse of the xb permutation: [2, 128, P] -> [C, P] f32."""
    o6 = op_.reshape(2, 16, 8, 8, 8, 64)  # (blk, wc, rb, rr, g, j)
    return o6.transpose(0, 4, 1, 2, 3, 5).reshape(C, P).astype(np.float32)


def kernel(x: np.ndarray, weight: np.ndarray) -> np.ndarray:
    x = np.ascontiguousarray(np.asarray(x, dtype=np.float32))
    weight = np.ascontiguousarray(np.asarray(weight, dtype=np.float32))
    assert x.shape == (B, C, H, W), x.shape
    assert weight.shape == (B, WC, K2, P), weight.shape

    nc = get_nc()
    in_maps = []
    for i in range(N_CORES):
        xi = x[i].reshape(C, P).astype(np.float16)
        in_maps.append(
            {
                "xb": _make_xb(xi),
                "xe": _make_xe(xi),
                "wt": _make_wt(weight[i]),
            }
        )
    try:
        res = run_bass_kernel_spmd(nc, in_maps, list(range(N_CORES)))
    except Exception:
        # the axon terminal occasionally reports a transient
        # NRT_EXEC_UNIT_UNRECOVERABLE for a known-good NEFF; retry once
        res = run_bass_kernel_spmd(nc, in_maps, list(range(N_CORES)))
    out = np.stack(
        [_unperm_out(res.results[i]["out"]) for i in range(N_CORES)], axis=0
    )
    return out.reshape(B, C, OH, OW)
